# revision 25
# baseline (speedup 1.0000x reference)
"""Trainium2 Bass kernel for nn_CoulombPotential (PhysNet-attenuated Coulomb energy).

Algorithm
---------
  per_system[s] = KE * sum_{pairs p: i<j, sys(i)=s} q[i] q[j] chi(d_p)
  chi(d) = phi(2d)/sqrt(d^2+1) + (1-phi(2d))/d,  phi = PhysNet switching fn.

Key observation: phi(2d) = 0 for d >= 0.5, so
  * HIGH branch (d >= 0.5, ~62% of pairs): chi = 1/d exactly, computed on the
    ACT engine as Exp(-Ln(d)) (both functions live in one activation table).
  * LOW branch (d < 0.5): chi(d) is smooth and bounded on (0, 0.5]; a degree-5
    polynomial fit reaches ~3e-4 abs error (tolerance is 2e-2).  Evaluated in
    two fused custom DVE ops (3 compile-time constants each).

Sharding / host marshalling (data movement only: mask, sort, gather, cast):
  * drop masked (i>=j) pairs, split each system's pairs into (low, high)
    blocks, serpentine-assign 128 systems to each of 8 cores balanced by pair
    count, pad each (system, branch) block to whole 256-slot rows,
  * streams d/qi/qj are sent as fp16 (6 B/pair vs 12 in f32); the row->system
    0/1 selector matrix is loaded once into SBUF outside the timed loop.

Device: qq=qi*qj on GPSIMD; chi on ACT (high) / custom DVE polys (low);
e=qq*chi fused with the per-row reduction in one tensor_tensor_reduce; the
rows->systems segment reduction as 0/1-selector matmuls accumulated in PSUM.
Core outputs are disjoint [128]-system slices; the host only concatenates.
"""
import functools

import numpy as np

import concourse.bacc as bacc
import concourse.bass_utils as bass_utils
import concourse.mybir as mybir
import concourse.tile as tile

F32 = mybir.dt.float32
F16 = mybir.dt.float16
AF = mybir.ActivationFunctionType
OP = mybir.AluOpType

KE = 138.96
N_CORES = 8
S_TOTAL = 1024
SYS_PER_CORE = S_TOTAL // N_CORES  # 128

PART = 128      # SBUF partitions
ROW = 256       # slots per logical row (system-block padding granularity)
CHUNK = PART    # rows per selector-matmul chunk (= partition count)
TILE_SUB_MAX = 6  # sub-rows (=row chunks) per full tile -> T = 1536

# Degree-5 minimax-ish fit of chi(d) on [0.045, 0.505] (Chebyshev nodes).
CHI_POLY = (-187.5327610377174, 420.17616084615247, -311.1689713054726,
            77.70598746001006, 0.1455691868852779, 0.9961215194616044)

# Row-chunk counts for the known dataset (max over cores, ceil to 128 rows).
# _host_marshal() recomputes them; _build_nc is parameterized so a different
# dataset would still work (at the cost of a recompile).
LOW_CHUNKS_DEFAULT = 13
HIGH_CHUNKS_DEFAULT = 21


def _tiles_for(low_chunks, high_chunks):
    """[(n_sub, region, chunk0), ...] with n_sub<=6 sub-rows of 256 slots.

    Low (DVE-heavy) and high (ACT-heavy) tiles are interleaved so the two
    engines' work overlaps instead of running as two serial phases."""
    tiles = []
    c0 = 0
    for region, n in (("L", low_chunks), ("H", high_chunks)):
        left = n
        while left > 0:
            take = min(TILE_SUB_MAX, left)
            tiles.append((take, region, c0))
            c0 += take
            left -= take
    return tuple(tiles)


@functools.lru_cache(maxsize=1)
def _register_chi_ops():
    """Three fused DVE ops:
       CHI_H1:  h = (d*s0 + s1)*d + imm2          (chi-poly Horner prefix)
       CHI_H2:  v = ((h*d + s0)*d + s1)*d + imm2  (chi-poly Horner finish)
       MUL_ACC: e = qq*chi; accum_out = sum(e)    (fused multiply + row-reduce)
    Registered via the documented OPS-append flow, sha pinned on the fly."""
    import concourse.dve_ops as dve_ops
    from concourse.dve_spec import Spec, Src0, Src1, C0, C1, C2, lower, AluOp
    from concourse.dve_uop import DveOpSpec

    names = ("CHI_H1", "CHI_H2", "MUL_ACC")
    have = {o.name: o for o in dve_ops.OPS if o.name in names}
    if len(have) == 3:
        return tuple(have[n] for n in names)

    def mk(name, body, ref):
        spec = Spec(body=body, reference=ref)
        shas = {v: DveOpSpec(name=name, opcode=1,
                             uops=lower(spec, ver=v)).sha(v) for v in ("v3", "v4")}
        op = dve_ops.DveOp(name, spec, subdim=False, uops_sha=shas)
        dve_ops.OPS.append(op)
        dve_ops.CUSTOM_DVE_SPECS[op.name] = op.spec
        dve_ops._SUB_OPCODE_FOR_NAME[op.name] = (
            dve_ops._CUSTOM_DVE_ROW_BASE + len(dve_ops.OPS) - 1)
        return op

    def mk2(name, spec):
        shas = {v: DveOpSpec(name=name, opcode=1,
                             uops=lower(spec, ver=v)).sha(v) for v in ("v3", "v4")}
        op = dve_ops.DveOp(name, spec, subdim=False, uops_sha=shas)
        dve_ops.OPS.append(op)
        dve_ops.CUSTOM_DVE_SPECS[op.name] = op.spec
        dve_ops._SUB_OPCODE_FOR_NAME[op.name] = (
            dve_ops._CUSTOM_DVE_ROW_BASE + len(dve_ops.OPS) - 1)
        return op

    h1 = mk("CHI_H1", (Src0 * C0 + C1) * Src0 + C2,
            lambda in0, s0, s1, imm2:
                ((in0 * s0 + s1) * in0 + imm2).astype(np.float32))
    h2 = mk("CHI_H2", ((Src1 * Src0 + C0) * Src0 + C1) * Src0 + C2,
            lambda in0, in1, s0, s1, imm2:
                ((((in1 * in0) + s0) * in0 + s1) * in0 + imm2).astype(np.float32))
    macc = mk2("MUL_ACC", Spec(body=Src0 * Src1, accum=AluOp.ADD,
                               reference=lambda in0, in1:
                                   (in0 * in1).astype(np.float32)))
    return h1, h2, macc


@functools.lru_cache(maxsize=4)
def _build_nc(repeat=0, low_chunks=LOW_CHUNKS_DEFAULT,
              high_chunks=HIGH_CHUNKS_DEFAULT):
    """repeat=0: straight-line kernel.  repeat=R>0: wrap the per-pair body in
    a hardware For_i loop (identical result; used by the test harness to
    measure per-iteration device time via slope)."""
    h1, h2, macc = _register_chi_ops()
    a5, a4, a3, a2, a1, a0 = CHI_POLY
    tiles = _tiles_for(low_chunks, high_chunks)
    n_chunks = low_chunks + high_chunks

    nc = bacc.Bacc("TRN2", target_bir_lowering=False, debug=False,
                   enable_asserts=False, num_devices=N_CORES)
    # one stream tensor per tile ([d | qi | qj] along the free dim); the
    # three thirds are DMA'd by three different issuing engines (SP, ACT,
    # GPSIMD) so their descriptor generation and transfers run concurrently
    # instead of serializing on the SP sequencer (~1.2us per issue).
    s_in = []
    for t, (nsub, region, c0) in enumerate(tiles):
        T = nsub * ROW
        s_in.append(nc.dram_tensor(f"s{t}", [PART, 3 * T], F16,
                                   kind="ExternalInput"))
    m_in = nc.dram_tensor("m_in", [PART, n_chunks, SYS_PER_CORE], F32,
                          kind="ExternalInput")
    out = nc.dram_tensor("out", [SYS_PER_CORE, 1], F32, kind="ExternalOutput")

    with tile.TileContext(nc) as tc:
        with (
            tc.tile_pool(name="io", bufs=7) as io,
            tc.tile_pool(name="tmp", bufs=5) as tmp,
            tc.tile_pool(name="sel", bufs=1) as sel,
            tc.tile_pool(name="acc", bufs=1) as acc,
            tc.tile_pool(name="psum", bufs=1, space="PSUM") as psp,
        ):
            ps = psp.tile([PART, 1], F32)
            # loop-invariant row->system selector, loaded once
            m_sb = sel.tile([PART, n_chunks, SYS_PER_CORE], F32, tag="m")
            nc.sync.dma_start(m_sb[:], m_in[:])

            def body():
                last_t = len(tiles) - 1
                for t, (nsub, region, c0) in enumerate(tiles):
                    T = nsub * ROW
                    st = io.tile([PART, 3 * T], F16, tag="st")
                    nc.sync.dma_start(st[:, 0:T], s_in[t][:, 0:T])
                    # alternate the qi issue between ACT and SP so neither
                    # sequencer's DMA-issue time stacks on its compute
                    qi_eng = nc.scalar if t % 2 == 0 else nc.sync
                    qi_eng.dma_start(st[:, T:2 * T], s_in[t][:, T:2 * T])
                    nc.gpsimd.dma_start(st[:, 2 * T:3 * T],
                                        s_in[t][:, 2 * T:3 * T])
                    d = st[:, 0:T]
                    qi = st[:, T:2 * T]
                    qj = st[:, 2 * T:3 * T]

                    qq = tmp.tile([PART, T], F16, tag="qq")
                    nc.gpsimd.tensor_tensor(qq[:], qi, qj, OP.mult)

                    if region == "L":
                        # h is ~[-300, -250]; keep it f32 so the Horner
                        # continuation doesn't amplify fp16 rounding of h.
                        hh = tmp.tile([PART, T], F32, tag="hh")
                        vv = tmp.tile([PART, T], F16, tag="vv")
                        nc.vector._custom_dve(h1, out=hh[:], in0=d,
                                              s0=a5, s1=a4, imm2=a3)
                        nc.vector._custom_dve(h2, out=vv[:], in0=d, in1=hh[:],
                                              s0=a2, s1=a1, imm2=a0)
                        src = vv
                    else:
                        lt = tmp.tile([PART, T], F16, tag="lt")
                        rv = tmp.tile([PART, T], F16, tag="rv")
                        nc.scalar.activation(lt[:], d, AF.Ln)
                        nc.scalar.activation(rv[:], lt[:], AF.Exp, scale=-1.0)
                        src = rv

                    ee = tmp.tile([PART, T], F16, tag="ee")
                    rsum = tmp.tile([PART, nsub], F32, tag="rsum")
                    for n in range(nsub):
                        sl = slice(n * ROW, (n + 1) * ROW)
                        nc.vector._custom_dve(
                            macc, out=ee[:, sl], in0=qq[:, sl],
                            in1=src[:, sl], accum_out=rsum[:, n:n + 1])
                    for n in range(nsub):
                        nc.tensor.matmul(ps[:], m_sb[:, c0 + n, :],
                                         rsum[:, n:n + 1],
                                         start=(t == 0 and n == 0),
                                         stop=(t == last_t and n == nsub - 1))

            if repeat > 0:
                with tc.For_i(0, repeat, 1):
                    body()
            else:
                body()
            res = acc.tile([SYS_PER_CORE, 1], F32, tag="res")
            nc.scalar.mul(res[:], ps[:], KE)
            nc.sync.dma_start(out[:], res[:])
    nc.compile()
    return nc


def _host_marshal(electrostatic_pair_indices, electrostatic_d_ij,
                  per_atom_charge, atomic_subsystem_indices):
    idx_i = np.asarray(electrostatic_pair_indices[0])
    idx_j = np.asarray(electrostatic_pair_indices[1])
    d = np.asarray(electrostatic_d_ij)[:, 0].astype(np.float32)
    q = np.asarray(per_atom_charge)[:, 0].astype(np.float32)
    sys_idx = np.asarray(atomic_subsystem_indices)

    keep = idx_i < idx_j
    ii = idx_i[keep]
    jj = idx_j[keep]
    dd = d[keep]
    seg = sys_idx[ii].astype(np.int64)
    hi = (dd >= 0.5).astype(np.int64)  # branch: phi(2d)=0 exactly for d>=0.5

    order = np.lexsort((hi, seg))      # by system, low-branch first
    ii, jj, dd, seg, hi = ii[order], jj[order], dd[order], seg[order], hi[order]

    # per (system, branch) block sizes; blocks padded to whole 256-slot rows
    blk = seg * 2 + hi                 # 2048 blocks
    counts_blk = np.bincount(blk, minlength=2 * S_TOTAL)
    counts_sys = np.bincount(seg, minlength=S_TOTAL)
    blk_start = np.concatenate([[0], np.cumsum(counts_blk)])

    # serpentine-assign systems (by descending total count) to cores
    order_sys = np.argsort(-counts_sys, kind="stable")
    k = np.arange(S_TOTAL)
    block_r, within = k // N_CORES, k % N_CORES
    core_of_rank = np.where(block_r % 2 == 0, within, N_CORES - 1 - within)
    sys_to_core = np.empty(S_TOTAL, np.int64)
    sys_to_core[order_sys] = core_of_rank
    sys_to_local = np.empty(S_TOTAL, np.int64)
    core_systems = np.empty((N_CORES, SYS_PER_CORE), np.int64)
    for c in range(N_CORES):
        mine = order_sys[core_of_rank == c]
        core_systems[c] = mine
        sys_to_local[mine] = np.arange(SYS_PER_CORE)

    rows_of_blk = -(-counts_blk // ROW)         # ceil
    # per-core per-region row layout (low region rows first, then high)
    rows_low_core = np.zeros(N_CORES, np.int64)
    rows_high_core = np.zeros(N_CORES, np.int64)
    for c in range(N_CORES):
        mine = core_systems[c]
        rows_low_core[c] = rows_of_blk[mine * 2].sum()
        rows_high_core[c] = rows_of_blk[mine * 2 + 1].sum()
    low_chunks = int(-(-rows_low_core.max() // CHUNK))
    high_chunks = int(-(-rows_high_core.max() // CHUNK))
    low_rows_pad = low_chunks * CHUNK
    n_chunks = low_chunks + high_chunks
    tot_rows = n_chunks * CHUNK
    slots = tot_rows * ROW

    # first row of each block within its core
    blk_row_base = np.zeros(2 * S_TOTAL, np.int64)
    for c in range(N_CORES):
        mine = core_systems[c]
        rb = np.concatenate([[0], np.cumsum(rows_of_blk[mine * 2])])
        blk_row_base[mine * 2] = rb[:-1]
        rb = np.concatenate([[0], np.cumsum(rows_of_blk[mine * 2 + 1])])
        blk_row_base[mine * 2 + 1] = low_rows_pad + rb[:-1]

    dest_core = sys_to_core[seg]
    dest_slot = (blk_row_base[blk] * ROW
                 + (np.arange(len(seg)) - blk_start[blk]))

    tiles = _tiles_for(low_chunks, high_chunks)

    in_maps = []
    for c in range(N_CORES):
        selm = dest_core == c
        dest = dest_slot[selm]
        dstream = np.empty(slots, np.float16)
        dstream[:low_rows_pad * ROW] = np.float16(0.25)   # low-branch pad
        dstream[low_rows_pad * ROW:] = np.float16(1.0)    # high-branch pad
        qis = np.zeros(slots, np.float16)
        qjs = np.zeros(slots, np.float16)
        dstream[dest] = dd[selm].astype(np.float16)
        qis[dest] = q[ii[selm]].astype(np.float16)
        qjs[dest] = q[jj[selm]].astype(np.float16)

        # 0/1 selector: row chunk c, partition p  ->  local system
        mine = core_systems[c]
        m = np.zeros((tot_rows, SYS_PER_CORE), np.float32)
        for reg in (0, 1):
            row_sys = np.repeat(sys_to_local[mine],
                                rows_of_blk[mine * 2 + reg])
            base = 0 if reg == 0 else low_rows_pad
            m[base + np.arange(len(row_sys)), row_sys] = 1.0
        m_dram = np.ascontiguousarray(
            m.reshape(n_chunks, CHUNK, SYS_PER_CORE).transpose(1, 0, 2))

        # streams: row r (global) = chunk*128 + partition; within a tile the
        # chunks are that tile's sub-rows: dram[p, n*256+k] = slot(row, k).
        # The three streams are fused as [d | qi | qj] along the free dim so
        # each tile is one DMA.
        per_core = {"m_in": m_dram}
        chunks_view = (dstream.reshape(n_chunks, CHUNK, ROW),
                       qis.reshape(n_chunks, CHUNK, ROW),
                       qjs.reshape(n_chunks, CHUNK, ROW))
        for t, (nsub, region, c0) in enumerate(tiles):
            parts = [arr[c0:c0 + nsub].transpose(1, 0, 2).reshape(
                PART, nsub * ROW) for arr in chunks_view]
            per_core[f"s{t}"] = np.ascontiguousarray(
                np.concatenate(parts, axis=1))
        in_maps.append(per_core)
    return in_maps, core_systems, low_chunks, high_chunks


def kernel(electrostatic_pair_indices, electrostatic_d_ij, per_atom_charge,
           atomic_subsystem_indices, num_systems):
    assert int(num_systems) == S_TOTAL
    in_maps, core_systems, low_chunks, high_chunks = _host_marshal(
        electrostatic_pair_indices, electrostatic_d_ij,
        per_atom_charge, atomic_subsystem_indices)
    nc = _build_nc(0, low_chunks, high_chunks)
    res = bass_utils.run_bass_kernel_spmd(nc, in_maps,
                                          core_ids=list(range(N_CORES)))
    full = np.empty(S_TOTAL, np.float32)
    for c in range(N_CORES):
        full[core_systems[c]] = res.results[c]["out"][:, 0]
    return full[:, None]
